# revision 49
# baseline (speedup 1.0000x reference)
"""Trainium2 Bass kernel for nn_Cffn_68478958568093 (dense_mlp).

out = x @ U_w.T + V(z),  z = a0 + continued_fraction(a[..,1:]),
a = (sigmoid(x @ gate_w.T) * x) @ ladder_w.T

Distribution: data-parallel over the 8192 tokens across 8 NeuronCores
(1024 tokens/core), weights replicated.  All on-chip compute is done in
feature-major (transposed) layout; the host transposes per-core shards in
and the final output back.

Precision budget (tolerance is rel 2e-2 of absmax 37.9 => abs ~0.76; the
continued fraction amplifies errors in `a` by ~1e4, and flips sign when
a-noise exceeds the min unguarded |1+f| ~ 4e-4 -- emulated margins in
emul2.py):
 - gate GEMM: fp16 hi*hi pass + fp8 DoubleRow chain for the w_lo * x
   cross term (weights prescaled x32).  Dropping only the w_hi*x_lo
   term; emulated max out err 0.232 (rel 6.1e-3, 3.3x margin).
 - ladder GEMM (K=2048 -> 18): fp16 hi*hi pass + one fp8 DoubleRow pass
   computing both cross terms (lw_hi*g_lo + lw_lo*g_hi), operands
   prescaled x32 to stay out of e4m3 subnormals => e_ladder ~ 4e-6.
 - U GEMM: tolerance-loose, fp8e4 DoubleRow (0.5 cyc/row, 256-row
   contraction per instr), weights prescaled x32; psum is divided by 32
   in the ACT-engine epilogue.
 - V (K=3): fp16 matmul appended to each U psum accumulation chain
   (V.T prescaled x32 to match), so the epilogue is a single ACT copy
   with scale 1/32 and there is no z broadcast or DVE FMA chain.

 - output written fp16 (adds <=2.4e-4 relative; halves the exit DMA).

PE work: gate fp16 109us + gate cross DR 27us, ladder 10us, U 27us,
V 7us => 182us busy; CoreSim ~199us total (lead-in, phase-boundary
pool barriers, exit drain).  Measured on HW: max abs err 0.232090
(rel 6.13e-3 vs the 2e-2 gate), CoreSim cost model 198803 ns vs the
478825 ns baseline (2.41x).
"""

import numpy as np
from contextlib import ExitStack

import concourse.bass as bass
import concourse.bacc as bacc
import concourse.mybir as mybir
import concourse.tile as tile
from concourse.bass_utils import run_bass_kernel_spmd
from concourse.masks import make_identity

NCORES = 8
D = 2048
TOKENS = 4 * 2048
TPC = TOKENS // NCORES      # tokens per core = 1024
KT = D // 128               # 16 contraction chunks
NDT = D // 128              # 16 output-row tiles
NTT = TPC // 128            # 8 token tiles of 128
L = 3
DEPTH = 5
LK = L * (DEPTH + 1)        # 18
EPS = 0.01
SC = 2048.0                 # 2^11 hi/lo split scale
W8 = 32.0                   # fp8 weight prescale (avoids e4m3 subnormals)
F16 = mybir.dt.float16
F32 = mybir.dt.float32
F8 = mybir.dt.float8e4
AOP = mybir.AluOpType
DR = mybir.MatmulPerfMode.DoubleRow


def _split16(a):
    """fp32 array -> (hi fp16, lo' fp16) with lo' = (a - hi) * 2048."""
    hi = a.astype(np.float16)
    lo = ((a - hi.astype(np.float32)) * SC).astype(np.float16)
    return hi, lo


def _to8(a):
    return np.clip(np.asarray(a, np.float32), -240.0, 240.0).astype(
        mybir.dt.np(F8))


def _build_program():
    nc = bacc.Bacc()

    def dp(name, shape, dt, out=False):
        return nc.declare_dram_parameter(name, list(shape), dt, isOutput=out)

    d_xhi = dp("xhi", [KT, 128, TPC], F16)
    d_xlo = dp("xlo", [KT, 128, TPC], F16)
    d_ghi = dp("ghi", [NDT, 128, KT, 128], F16)   # [dt][p][k][o]
    d_gl8 = dp("gl8", [NDT, 128, KT, 128], F8)    # gate_lo16 * 32, same tiling
    d_u8 = dp("u8", [NDT, 128, KT, 128], F8)      # U_w * 32, same tiling
    d_lwhi = dp("lwhi", [128, KT, LK], F16)
    # (lw_hi*32, lw_lo*32) pairs; last dim padded 18->32 so the DoubleRow
    # LDWEIGHTS subtile step is a multiple of 16 (ISA requirement)
    d_lwp8 = dp("lwp8", [128, KT, 2, 32], F8)
    d_vw = dp("vwT", [L, NDT, 128], F16)          # V_w.T * 32
    d_vsc = dp("vsc", [128, NDT, L], F32)         # V_w rows by partition
    d_out = dp("outT", [D, TPC], F16, out=True)

    with tile.TileContext(nc) as tc, ExitStack() as ctx:
        persist = ctx.enter_context(tc.tile_pool(name="persist", bufs=1))
        gwp = ctx.enter_context(tc.tile_pool(name="gw", bufs=2))
        drp = ctx.enter_context(tc.tile_pool(name="drs", bufs=1, space="DRAM"))

        # dt0's gate weights load at the head of both DMA queues so the PE
        # can start within ~1us; the first xhi tile is split for the same
        # reason.  x8 (fp8 x for the DoubleRow chains) is cast on-chip by
        # the DVE as each xhi tile lands -- no DMA on the critical lead-in.
        gh0 = gwp.tile([128, KT, 128], F16, tag="gh")
        nc.sync.dma_start(out=gh0[:, 0:2, :], in_=d_ghi[0][:, 0:2, :])
        gl80 = gwp.tile([128, KT, 128], F8, tag="gl8")
        nc.scalar.dma_start(out=gl80, in_=d_gl8[:, :, :, :][0])
        x8t = persist.tile([128, KT, TPC], F8, tag="x8t")
        xhi, xlo = [], []
        for k in range(KT):
            t = persist.tile([128, TPC], F16, tag=f"xhi{k}")
            if k == 0:
                nc.sync.dma_start(out=t[:, 0:512], in_=d_xhi[0][:, 0:512])
                nc.sync.dma_start(out=t[:, 512:TPC], in_=d_xhi[0][:, 512:TPC])
                nc.sync.dma_start(out=gh0[:, 2:KT, :], in_=d_ghi[0][:, 2:KT, :])
            else:
                nc.sync.dma_start(out=t, in_=d_xhi[:, :, :][k])
            nc.vector.tensor_copy(x8t[:, k, :], t)
            xhi.append(t)
        for k in range(KT):
            t = persist.tile([128, TPC], F16, tag=f"xlo{k}")
            nc.sync.dma_start(out=t, in_=d_xlo[:, :, :][k])
            xlo.append(t)

        lwhi = persist.tile([128, KT, LK], F16, tag="lwhi")
        lwp8 = persist.tile([128, KT, 2, 32], F8, tag="lwp8")
        vw = persist.tile([L, NDT, 128], F16, tag="vw")
        vsc = persist.tile([128, NDT, L], F32, tag="vsc")

        ident = persist.tile([128, 128], F32, tag="ident")
        make_identity(nc, ident)

        ghi_g = []                  # gated_x hi fp16, by row tile
        for k in range(KT):
            ghi_g.append(persist.tile([128, TPC], F16, name=f"gghi{k}", tag=f"gghi{k}"))
        # fp8 (lo, hi) pairs of gated_x for the ladder cross-term DoubleRow
        gq8 = persist.tile([128, KT, 2, TPC], F8, tag="gq8")
        zt = persist.tile([128, NTT, L], F32, tag="zt")
        zT16 = persist.tile([L, TPC], F16, tag="zT16")

        # ---------------- Phase A: gated_x = sigmoid(x @ gate_w.T) * x -----
        # fp16 hi*hi pass + fp8 DoubleRow chain for the w_lo*x cross term;
        # epilogue reconstructs x in fp32, applies the sigmoid gate, and
        # stores gated_x both as fp16 hi and as fp8 (lo, hi) pairs.
        with tc.tile_pool(name="psA", bufs=2, space="PSUM") as psA, \
             tc.tile_pool(name="epi", bufs=1) as epi:
            for dt in range(NDT):
                if dt == 0:
                    gh, gl8 = gh0, gl80
                else:
                    gh = gwp.tile([128, KT, 128], F16, tag="gh")
                    nc.scalar.dma_start(out=gh, in_=d_ghi[:, :, :, :][dt])
                    gl8 = gwp.tile([128, KT, 128], F8, tag="gl8")
                    nc.scalar.dma_start(out=gl8, in_=d_gl8[:, :, :, :][dt])
                if dt == 1:
                    # small phase-B/C constants ride behind dt1's weights
                    nc.scalar.dma_start(out=lwhi, in_=d_lwhi[:, :, :])
                    nc.scalar.dma_start(out=lwp8, in_=d_lwp8[:, :, :, :])
                    nc.scalar.dma_start(out=vw, in_=d_vw[:, :, :])
                    nc.scalar.dma_start(out=vsc, in_=d_vsc[:, :, :])

                pm = [psA.tile([128, 512], F32, name=f"pm{dt}_{t}", tag=f"pm{t}") for t in range(2)]
                pc = [psA.tile([128, 512], F32, name=f"pc{dt}_{t}", tag=f"pc{t}") for t in range(2)]
                for k in range(KT):
                    first, last = k == 0, k == KT - 1
                    for t in range(2):
                        sl = slice(t * 512, (t + 1) * 512)
                        nc.tensor.matmul(pm[t], gh[:, k, :], xhi[k][:, sl],
                                         start=first, stop=last)
                for kk in range(KT // 2):
                    first, last = kk == 0, kk == KT // 2 - 1
                    for t in range(2):
                        sl = slice(t * 512, (t + 1) * 512)
                        nc.tensor.matmul(
                            pc[t], gl8[:, 2 * kk:2 * kk + 2, :],
                            x8t[:, 2 * kk:2 * kk + 2, sl],
                            start=first, stop=last, perf_mode=DR)

                # DVE may read only one PSUM operand per op: ACT stages the
                # scaled cross psum to SBUF, DVE adds the hi*hi psum.
                c32 = epi.tile([128, TPC], F32, tag="c32")
                y32 = epi.tile([128, TPC], F32, tag="y32")
                sig = epi.tile([128, TPC], F32, tag="sig")
                for t in range(2):
                    sl = slice(t * 512, (t + 1) * 512)
                    nc.scalar.activation(c32[:, sl], pc[t],
                                         mybir.ActivationFunctionType.Copy,
                                         scale=1.0 / (SC * W8))
                    nc.vector.tensor_add(y32[:, sl], c32[:, sl], pm[t])
                    nc.scalar.activation(sig[:, sl], y32[:, sl],
                                         mybir.ActivationFunctionType.Sigmoid)
                x32 = epi.tile([128, TPC], F32, tag="x32")
                nc.vector.scalar_tensor_tensor(
                    out=x32, in0=xlo[dt], scalar=1.0 / SC, in1=xhi[dt],
                    op0=AOP.mult, op1=AOP.add)
                g32 = epi.tile([128, TPC], F32, tag="g32")
                nc.vector.tensor_mul(g32, sig, x32)
                nc.vector.tensor_copy(ghi_g[dt], g32)
                d32 = epi.tile([128, TPC], F32, tag="d32")
                nc.vector.scalar_tensor_tensor(
                    out=d32, in0=ghi_g[dt], scalar=-1.0, in1=g32,
                    op0=AOP.mult, op1=AOP.add)
                # fp8 lo (scaled by SC) and hi copies for the ladder cross
                nc.vector.tensor_scalar_mul(gq8[:, dt, 0, :], d32, SC)
                nc.vector.tensor_copy(gq8[:, dt, 1, :], ghi_g[dt])

        # ---------------- Phase B: a = gated @ lw.T ; CF ; z ---------------
        # hi*hi in fp16 + both cross terms in one fp8 DoubleRow chain.
        with tc.tile_pool(name="cfb", bufs=1) as cfb, \
             tc.tile_pool(name="psB", bufs=2, space="PSUM") as psB:
            a32 = cfb.tile([LK, TPC], F32, tag="a32")
            for t in range(2):
                sl = slice(t * 512, (t + 1) * 512)
                pam = psB.tile([LK, 512], F32, tag="pam")
                pac = psB.tile([LK, 512], F32, tag="pac")
                for k in range(KT):
                    first, last = k == 0, k == KT - 1
                    nc.tensor.matmul(pam, lwhi[:, k, :], ghi_g[k][:, sl],
                                     start=first, stop=last)
                for k in range(KT):
                    first, last = k == 0, k == KT - 1
                    nc.tensor.matmul(pac, lwp8[:, k, :, 0:LK], gq8[:, k, :, sl],
                                     start=first, stop=last, perf_mode=DR)
                nc.vector.tensor_copy(a32[:, sl], pam)
                nc.vector.scalar_tensor_tensor(
                    out=a32[:, sl], in0=pac, scalar=1.0 / (SC * W8),
                    in1=a32[:, sl], op0=AOP.mult, op1=AOP.add)

            # transpose a to token-major [128, tt, l, k]
            at = cfb.tile([128, NTT, L, DEPTH + 1], F32, tag="at")
            for tt in range(NTT):
                pt = psB.tile([128, LK], F32, tag="pt")
                nc.tensor.transpose(
                    pt, a32[:, tt * 128:(tt + 1) * 128], ident[:LK, :LK])
                nc.vector.tensor_copy(
                    at[:, tt, :, :].rearrange("p l k -> p (l k)"), pt)

            # continued fraction with eps-guarded denominators
            f = cfb.tile([128, NTT, L], F32, tag="f")
            t1 = cfb.tile([128, NTT, L], F32, tag="t1")
            dc = cfb.tile([128, NTT, L], F32, tag="dc")
            msk = cfb.tile([128, NTT, L], mybir.dt.uint8, tag="msk")
            rc = cfb.tile([128, NTT, L], F32, tag="rc")
            nc.vector.tensor_copy(f, at[:, :, :, DEPTH])
            for kk in range(DEPTH - 1, 0, -1):
                nc.vector.tensor_scalar(out=t1, in0=f, scalar1=1.0,
                                        scalar2=EPS, op0=AOP.add, op1=AOP.max)
                nc.vector.tensor_scalar(out=dc, in0=f, scalar1=1.0,
                                        scalar2=-EPS, op0=AOP.add, op1=AOP.min)
                nc.vector.tensor_scalar(out=msk, in0=f, scalar1=1.0,
                                        scalar2=0.0, op0=AOP.add, op1=AOP.is_ge)
                nc.vector.copy_predicated(dc, msk, t1)
                nc.vector.reciprocal(rc, dc)
                nc.vector.tensor_mul(f, at[:, :, :, kk], rc)
            nc.vector.tensor_add(zt, at[:, :, :, 0], f)

        # ---------------- Phase C: out = (32*(x@U.T) + 32*(z@V.T)) / 32 ----
        # U in fp8 DoubleRow; V appended as a K=3 fp16 matmul on the same
        # psum chain once z is ready; epilogue is one ACT copy w/ scale.
        # The first NSTAGE chains stop without V and stage scaled fp16
        # U-results to SBUF, freeing psum so the PE streams through the
        # continued-fraction latency window; their V matmuls land in fresh
        # psum later and an fp16 DVE add (2x mode) merges the halves.
        NSTAGE = 0
        with tc.tile_pool(name="uw", bufs=3) as uwp, \
             tc.tile_pool(name="psC", bufs=3, space="PSUM") as psC, \
             tc.tile_pool(name="u16p", bufs=1) as u16p, \
             tc.tile_pool(name="ob", bufs=2) as obp:
            def emit_c_mms(dt, stop):
                ut = uwp.tile([128, KT, 128], F8, name=f"ut{dt}", tag="ut")
                # u8 weights ride the sync queue: SP is idle in phase C while
                # ACT is still draining phase A epilogues
                nc.sync.dma_start(out=ut, in_=d_u8[:, :, :, :][dt])
                po = [psC.tile([128, 512], F32, name=f"po{dt}_{t}", tag=f"po{t}") for t in range(2)]
                for kk in range(KT // 2):
                    for t in range(2):
                        sl = slice(t * 512, (t + 1) * 512)
                        nc.tensor.matmul(
                            po[t], ut[:, 2 * kk:2 * kk + 2, :],
                            x8t[:, 2 * kk:2 * kk + 2, sl],
                            start=(kk == 0), stop=(stop and kk == KT // 2 - 1),
                            perf_mode=DR)
                return po

            def emit_c_epilogue(dt, po, nq=1):
                o16 = obp.tile([128, TPC], F16, name=f"o16_{dt}", tag="o16")
                for t in range(2):
                    sl = slice(t * 512, (t + 1) * 512)
                    nc.tensor.matmul(po[t], vw[:, dt, :], zT16[:, sl],
                                     start=False, stop=True)
                    for q in range(nq):
                        qs = slice(t * 512 + q * (512 // nq),
                                   t * 512 + (q + 1) * (512 // nq))
                        qp = slice(q * (512 // nq), (q + 1) * (512 // nq))
                        nc.scalar.activation(o16[:, qs], po[t][:, qp],
                                             mybir.ActivationFunctionType.Copy,
                                             scale=1.0 / W8)
                        nc.sync.dma_start(
                            out=d_out[dt * 128:(dt + 1) * 128, qs],
                            in_=o16[:, qs])

            pend = [emit_c_mms(dt, stop=False) for dt in range(3)]
            for tt in range(NTT):
                pz = psC.tile([L, 128], F32, name=f"pz{tt}", tag="pz", bufs=2)
                nc.tensor.transpose(pz, zt[:, tt, :], ident)
                nc.vector.tensor_copy(zT16[:, tt * 128:(tt + 1) * 128], pz)
            for dt in range(3):
                emit_c_epilogue(dt, pend[dt])
            for dt in range(3, NDT):
                po = emit_c_mms(dt, stop=False)
                emit_c_epilogue(dt, po, nq=2 if dt == NDT - 1 else 1)

    nc.finalize()
    return nc


_NC_CACHE = {}


def _get_program():
    if "nc" not in _NC_CACHE:
        _NC_CACHE["nc"] = _build_program()
    return _NC_CACHE["nc"]


def make_in_maps(x, U_w, gate_w, ladder_w, V_w):
    """Host-side sharding + layout prep. Returns per-core input dicts."""
    x2 = np.ascontiguousarray(np.asarray(x, dtype=np.float32).reshape(TOKENS, D))

    def wtiles(w):
        # w: [out, in] fp32 -> tiles [dt][p][k][o] with
        # tile[dt, p, k, o] = w[dt*128+o, k*128+p]
        wT = w.T.astype(np.float32)                    # [d, o]
        a = wT.reshape(KT, 128, NDT, 128)              # [k, p, dt, o]
        return np.ascontiguousarray(a.transpose(2, 1, 0, 3))

    U_w = np.asarray(U_w, np.float32)
    gate_w = np.asarray(gate_w, np.float32)
    ladder_w = np.asarray(ladder_w, np.float32)
    V_w = np.asarray(V_w, np.float32)

    g_tiles = wtiles(gate_w)
    ghi_t = g_tiles.astype(np.float16)
    glo_t = (g_tiles - ghi_t.astype(np.float32)) * SC
    gl8_t = _to8(glo_t * W8)
    u8_t = _to8(wtiles(U_w) * W8)

    lwT = ladder_w.transpose(2, 0, 1).reshape(D, LK)   # [d, (l k)]
    lw_hi, lw_lo = _split16(lwT)
    # [p, k, lk] with element (p,k,lk) = lwT[k*128+p, lk]
    lwhi_t = np.ascontiguousarray(
        lw_hi.reshape(KT, 128, LK).transpose(1, 0, 2))
    lwp8_t = np.zeros((128, KT, 2, 32), dtype=mybir.dt.np(F8))
    lwp8_t[:, :, 0, :LK] = _to8(
        lw_hi.astype(np.float32).reshape(KT, 128, LK).transpose(1, 0, 2) * W8)
    lwp8_t[:, :, 1, :LK] = _to8(
        lw_lo.astype(np.float32).reshape(KT, 128, LK).transpose(1, 0, 2) * W8)

    vwT = np.ascontiguousarray(
        (V_w.T.reshape(L, NDT, 128) * W8).astype(np.float16))
    vsc_t = np.ascontiguousarray(
        V_w.reshape(NDT, 128, L).transpose(1, 0, 2)).astype(np.float32)

    in_maps = []
    for c in range(NCORES):
        shard = x2[c * TPC:(c + 1) * TPC]              # [TPC, D]
        xT = np.ascontiguousarray(shard.T)             # [D, TPC]
        x_hi, x_lo = _split16(xT)
        in_maps.append({
            "xhi": np.ascontiguousarray(x_hi.reshape(KT, 128, TPC)),
            "xlo": np.ascontiguousarray(x_lo.reshape(KT, 128, TPC)),
            "ghi": ghi_t, "gl8": gl8_t, "u8": u8_t,
            "lwhi": lwhi_t, "lwp8": lwp8_t, "vwT": vwT, "vsc": vsc_t,
        })
    return in_maps


def assemble_output(results):
    parts = [results[c]["outT"].astype(np.float32).T
             for c in range(NCORES)]                         # [TPC, D] each
    out = np.concatenate(parts, axis=0)                      # [TOKENS, D]
    return np.ascontiguousarray(out.reshape(4, 2048, D).astype(np.float32))


def kernel(x, U_w, gate_w, ladder_w, V_w):
    nc = _get_program()
    in_maps = make_in_maps(x, U_w, gate_w, ladder_w, V_w)
    res = run_bass_kernel_spmd(nc, in_maps, list(range(NCORES)))
    return assemble_output(res.results)


if __name__ == "__main__":
    rng = np.random.default_rng(0)
    x = rng.normal(0, 1, (4, 2048, D)).astype(np.float32)
    s = 1.0 / np.sqrt(D)
    U_w = rng.uniform(-s, s, (D, D)).astype(np.float32)
    gate_w = rng.uniform(-s, s, (D, D)).astype(np.float32)
    ladder_w = rng.uniform(-s, s, (L, DEPTH + 1, D)).astype(np.float32)
    V_w = rng.uniform(-1 / np.sqrt(L), 1 / np.sqrt(L), (D, L)).astype(np.float32)
    out = kernel(x=x, U_w=U_w, gate_w=gate_w, ladder_w=ladder_w, V_w=V_w)
    print("out", out.shape, out.dtype, np.abs(out).max())


# revision 53
# speedup vs baseline: 1.0055x; 1.0055x over previous
"""Trainium2 Bass kernel for nn_Cffn_68478958568093 (dense_mlp).

out = x @ U_w.T + V(z),  z = a0 + continued_fraction(a[..,1:]),
a = (sigmoid(x @ gate_w.T) * x) @ ladder_w.T

Distribution: data-parallel over the 8192 tokens across 8 NeuronCores
(1024 tokens/core), weights replicated.  All on-chip compute is done in
feature-major (transposed) layout; the host transposes per-core shards in
and the final output back.

Precision budget (tolerance is rel 2e-2 of absmax 37.9 => abs ~0.76; the
continued fraction amplifies errors in `a` by ~1e4, and flips sign when
a-noise exceeds the min unguarded |1+f| ~ 4e-4 -- emulated margins in
emul2.py):
 - gate GEMM: fp16 hi*hi pass + fp8 DoubleRow chain for the w_lo * x
   cross term (weights prescaled x32).  Dropping only the w_hi*x_lo
   term; emulated max out err 0.232 (rel 6.1e-3, 3.3x margin).
 - ladder GEMM (K=2048 -> 18): fp16 hi*hi pass + one fp8 DoubleRow pass
   computing both cross terms (lw_hi*g_lo + lw_lo*g_hi), operands
   prescaled x32 to stay out of e4m3 subnormals => e_ladder ~ 4e-6.
 - U GEMM: tolerance-loose, fp8e4 DoubleRow (0.5 cyc/row, 256-row
   contraction per instr), weights prescaled x32; psum is divided by 32
   in the ACT-engine epilogue.
 - V (K=3): fp16 matmul appended to each U psum accumulation chain
   (V.T prescaled x32 to match), so the epilogue is a single ACT copy
   with scale 1/32 and there is no z broadcast or DVE FMA chain.

 - output written fp16 (adds <=2.4e-4 relative; halves the exit DMA).

PE work: gate fp16 109us + gate cross DR 27us, ladder 10us, U 27us,
V 7us => 182us busy; CoreSim ~199us total (lead-in, phase-boundary
pool barriers, exit drain).  Measured on HW: max abs err 0.232090
(rel 6.13e-3 vs the 2e-2 gate), CoreSim cost model 198803 ns vs the
478825 ns baseline (2.41x).
"""

import numpy as np
from contextlib import ExitStack

import concourse.bass as bass
import concourse.bacc as bacc
import concourse.mybir as mybir
import concourse.tile as tile
from concourse.bass_utils import run_bass_kernel_spmd
from concourse.masks import make_identity

NCORES = 8
D = 2048
TOKENS = 4 * 2048
TPC = TOKENS // NCORES      # tokens per core = 1024
KT = D // 128               # 16 contraction chunks
NDT = D // 128              # 16 output-row tiles
NTT = TPC // 128            # 8 token tiles of 128
L = 3
DEPTH = 5
LK = L * (DEPTH + 1)        # 18
EPS = 0.01
SC = 2048.0                 # 2^11 hi/lo split scale
W8 = 32.0                   # fp8 weight prescale (avoids e4m3 subnormals)
F16 = mybir.dt.float16
F32 = mybir.dt.float32
F8 = mybir.dt.float8e4
AOP = mybir.AluOpType
DR = mybir.MatmulPerfMode.DoubleRow


def _split16(a):
    """fp32 array -> (hi fp16, lo' fp16) with lo' = (a - hi) * 2048."""
    hi = a.astype(np.float16)
    lo = ((a - hi.astype(np.float32)) * SC).astype(np.float16)
    return hi, lo


def _to8(a):
    return np.clip(np.asarray(a, np.float32), -240.0, 240.0).astype(
        mybir.dt.np(F8))


def _build_program():
    nc = bacc.Bacc()

    def dp(name, shape, dt, out=False):
        return nc.declare_dram_parameter(name, list(shape), dt, isOutput=out)

    d_xhi = dp("xhi", [KT, 128, TPC], F16)
    d_xlo = dp("xlo", [KT, 128, TPC], F16)
    d_ghi = dp("ghi", [NDT, 128, KT, 128], F16)   # [dt][p][k][o]
    d_gl8 = dp("gl8", [NDT, 128, KT, 128], F8)    # gate_lo16 * 32, same tiling
    d_u8 = dp("u8", [NDT, 128, KT, 128], F8)      # U_w * 32, same tiling
    d_lwhi = dp("lwhi", [128, KT, LK], F16)
    # (lw_hi*32, lw_lo*32) pairs; last dim padded 18->32 so the DoubleRow
    # LDWEIGHTS subtile step is a multiple of 16 (ISA requirement)
    d_lwp8 = dp("lwp8", [128, KT, 2, 32], F8)
    d_vw = dp("vwT", [L, NDT, 128], F16)          # V_w.T * 32
    d_vsc = dp("vsc", [128, NDT, L], F32)         # V_w rows by partition
    d_out = dp("outT", [D, TPC], F16, out=True)

    with tile.TileContext(nc) as tc, ExitStack() as ctx:
        persist = ctx.enter_context(tc.tile_pool(name="persist", bufs=1))
        gwp = ctx.enter_context(tc.tile_pool(name="gw", bufs=2))
        drp = ctx.enter_context(tc.tile_pool(name="drs", bufs=1, space="DRAM"))

        # dt0's gate weights load at the head of both DMA queues so the PE
        # can start within ~1us; the first xhi tile is split for the same
        # reason.  x8 (fp8 x for the DoubleRow chains) is cast on-chip by
        # the DVE as each xhi tile lands -- no DMA on the critical lead-in.
        gh0 = gwp.tile([128, KT, 128], F16, tag="gh")
        nc.sync.dma_start(out=gh0[:, 0:2, :], in_=d_ghi[0][:, 0:2, :])
        # the bulk of dt0's weights rides the scalar queue so it does not
        # stall the xhi stream on sync (k=2 is needed ~3.4us in)
        nc.scalar.dma_start(out=gh0[:, 2:KT, :], in_=d_ghi[0][:, 2:KT, :])
        gl80 = gwp.tile([128, KT, 128], F8, tag="gl8")
        nc.scalar.dma_start(out=gl80, in_=d_gl8[:, :, :, :][0])
        x8t = persist.tile([128, KT, TPC], F8, tag="x8t")
        xhi, xlo = [], []
        for k in range(KT):
            t = persist.tile([128, TPC], F16, tag=f"xhi{k}")
            if k == 0:
                nc.sync.dma_start(out=t[:, 0:512], in_=d_xhi[0][:, 0:512])
                nc.sync.dma_start(out=t[:, 512:TPC], in_=d_xhi[0][:, 512:TPC])
            else:
                nc.sync.dma_start(out=t, in_=d_xhi[:, :, :][k])
            nc.vector.tensor_copy(x8t[:, k, :], t)
            xhi.append(t)
        for k in range(KT):
            t = persist.tile([128, TPC], F16, tag=f"xlo{k}")
            nc.sync.dma_start(out=t, in_=d_xlo[:, :, :][k])
            xlo.append(t)

        lwhi = persist.tile([128, KT, LK], F16, tag="lwhi")
        lwp8 = persist.tile([128, KT, 2, 32], F8, tag="lwp8")
        vw = persist.tile([L, NDT, 128], F16, tag="vw")
        vsc = persist.tile([128, NDT, L], F32, tag="vsc")

        ident = persist.tile([128, 128], F32, tag="ident")
        make_identity(nc, ident)

        ghi_g = []                  # gated_x hi fp16, by row tile
        for k in range(KT):
            ghi_g.append(persist.tile([128, TPC], F16, name=f"gghi{k}", tag=f"gghi{k}"))
        # fp8 (lo, hi) pairs of gated_x for the ladder cross-term DoubleRow
        gq8 = persist.tile([128, KT, 2, TPC], F8, tag="gq8")
        zt = persist.tile([128, NTT, L], F32, tag="zt")
        zT16 = persist.tile([L, TPC], F16, tag="zT16")

        # ---------------- Phase A: gated_x = sigmoid(x @ gate_w.T) * x -----
        # fp16 hi*hi pass + fp8 DoubleRow chain for the w_lo*x cross term;
        # epilogue reconstructs x in fp32, applies the sigmoid gate, and
        # stores gated_x both as fp16 hi and as fp8 (lo, hi) pairs.
        with tc.tile_pool(name="psA", bufs=2, space="PSUM") as psA, \
             tc.tile_pool(name="epi", bufs=1) as epi:
            for dt in range(NDT):
                if dt == 0:
                    gh, gl8 = gh0, gl80
                else:
                    gh = gwp.tile([128, KT, 128], F16, tag="gh")
                    nc.scalar.dma_start(out=gh, in_=d_ghi[:, :, :, :][dt])
                    gl8 = gwp.tile([128, KT, 128], F8, tag="gl8")
                    nc.scalar.dma_start(out=gl8, in_=d_gl8[:, :, :, :][dt])
                if dt == 1:
                    # small phase-B/C constants ride behind dt1's weights
                    nc.scalar.dma_start(out=lwhi, in_=d_lwhi[:, :, :])
                    nc.scalar.dma_start(out=lwp8, in_=d_lwp8[:, :, :, :])
                    nc.scalar.dma_start(out=vw, in_=d_vw[:, :, :])
                    nc.scalar.dma_start(out=vsc, in_=d_vsc[:, :, :])

                pm = [psA.tile([128, 512], F32, name=f"pm{dt}_{t}", tag=f"pm{t}") for t in range(2)]
                pc = [psA.tile([128, 512], F32, name=f"pc{dt}_{t}", tag=f"pc{t}") for t in range(2)]
                for k in range(KT):
                    first, last = k == 0, k == KT - 1
                    for t in range(2):
                        sl = slice(t * 512, (t + 1) * 512)
                        nc.tensor.matmul(pm[t], gh[:, k, :], xhi[k][:, sl],
                                         start=first, stop=last)
                for kk in range(KT // 2):
                    first, last = kk == 0, kk == KT // 2 - 1
                    for t in range(2):
                        sl = slice(t * 512, (t + 1) * 512)
                        nc.tensor.matmul(
                            pc[t], gl8[:, 2 * kk:2 * kk + 2, :],
                            x8t[:, 2 * kk:2 * kk + 2, sl],
                            start=first, stop=last, perf_mode=DR)

                # DVE may read only one PSUM operand per op: ACT stages the
                # scaled cross psum to SBUF, DVE adds the hi*hi psum.
                c32 = epi.tile([128, TPC], F32, tag="c32")
                y32 = epi.tile([128, TPC], F32, tag="y32")
                sig = epi.tile([128, TPC], F32, tag="sig")
                for t in range(2):
                    sl = slice(t * 512, (t + 1) * 512)
                    nc.scalar.activation(c32[:, sl], pc[t],
                                         mybir.ActivationFunctionType.Copy,
                                         scale=1.0 / (SC * W8))
                    nc.vector.tensor_add(y32[:, sl], c32[:, sl], pm[t])
                    nc.scalar.activation(sig[:, sl], y32[:, sl],
                                         mybir.ActivationFunctionType.Sigmoid)
                x32 = epi.tile([128, TPC], F32, tag="x32")
                nc.vector.scalar_tensor_tensor(
                    out=x32, in0=xlo[dt], scalar=1.0 / SC, in1=xhi[dt],
                    op0=AOP.mult, op1=AOP.add)
                g32 = epi.tile([128, TPC], F32, tag="g32")
                nc.vector.tensor_mul(g32, sig, x32)
                nc.vector.tensor_copy(ghi_g[dt], g32)
                d32 = epi.tile([128, TPC], F32, tag="d32")
                nc.vector.scalar_tensor_tensor(
                    out=d32, in0=ghi_g[dt], scalar=-1.0, in1=g32,
                    op0=AOP.mult, op1=AOP.add)
                # fp8 lo (scaled by SC) and hi copies for the ladder cross
                nc.vector.tensor_scalar_mul(gq8[:, dt, 0, :], d32, SC)
                nc.vector.tensor_copy(gq8[:, dt, 1, :], ghi_g[dt])

        # ---------------- Phase B: a = gated @ lw.T ; CF ; z ---------------
        # hi*hi in fp16 + both cross terms in one fp8 DoubleRow chain.
        with tc.tile_pool(name="cfb", bufs=1) as cfb, \
             tc.tile_pool(name="psB", bufs=2, space="PSUM") as psB:
            a32 = cfb.tile([LK, TPC], F32, tag="a32")
            for t in range(2):
                sl = slice(t * 512, (t + 1) * 512)
                pam = psB.tile([LK, 512], F32, tag="pam")
                pac = psB.tile([LK, 512], F32, tag="pac")
                for k in range(KT):
                    first, last = k == 0, k == KT - 1
                    nc.tensor.matmul(pam, lwhi[:, k, :], ghi_g[k][:, sl],
                                     start=first, stop=last)
                for k in range(KT):
                    first, last = k == 0, k == KT - 1
                    nc.tensor.matmul(pac, lwp8[:, k, :, 0:LK], gq8[:, k, :, sl],
                                     start=first, stop=last, perf_mode=DR)
                nc.vector.tensor_copy(a32[:, sl], pam)
                nc.vector.scalar_tensor_tensor(
                    out=a32[:, sl], in0=pac, scalar=1.0 / (SC * W8),
                    in1=a32[:, sl], op0=AOP.mult, op1=AOP.add)

            # transpose a to token-major [128, tt, l, k]
            at = cfb.tile([128, NTT, L, DEPTH + 1], F32, tag="at")
            for tt in range(NTT):
                pt = psB.tile([128, LK], F32, tag="pt")
                nc.tensor.transpose(
                    pt, a32[:, tt * 128:(tt + 1) * 128], ident[:LK, :LK])
                nc.vector.tensor_copy(
                    at[:, tt, :, :].rearrange("p l k -> p (l k)"), pt)

            # continued fraction with eps-guarded denominators
            f = cfb.tile([128, NTT, L], F32, tag="f")
            t1 = cfb.tile([128, NTT, L], F32, tag="t1")
            dc = cfb.tile([128, NTT, L], F32, tag="dc")
            msk = cfb.tile([128, NTT, L], mybir.dt.uint8, tag="msk")
            rc = cfb.tile([128, NTT, L], F32, tag="rc")
            nc.vector.tensor_copy(f, at[:, :, :, DEPTH])
            for kk in range(DEPTH - 1, 0, -1):
                nc.vector.tensor_scalar(out=t1, in0=f, scalar1=1.0,
                                        scalar2=EPS, op0=AOP.add, op1=AOP.max)
                nc.vector.tensor_scalar(out=dc, in0=f, scalar1=1.0,
                                        scalar2=-EPS, op0=AOP.add, op1=AOP.min)
                nc.vector.tensor_scalar(out=msk, in0=f, scalar1=1.0,
                                        scalar2=0.0, op0=AOP.add, op1=AOP.is_ge)
                nc.vector.copy_predicated(dc, msk, t1)
                nc.vector.reciprocal(rc, dc)
                nc.vector.tensor_mul(f, at[:, :, :, kk], rc)
            nc.vector.tensor_add(zt, at[:, :, :, 0], f)

        # ---------------- Phase C: out = (32*(x@U.T) + 32*(z@V.T)) / 32 ----
        # U in fp8 DoubleRow; V appended as a K=3 fp16 matmul on the same
        # psum chain once z is ready; epilogue is one ACT copy w/ scale.
        # The first NSTAGE chains stop without V and stage scaled fp16
        # U-results to SBUF, freeing psum so the PE streams through the
        # continued-fraction latency window; their V matmuls land in fresh
        # psum later and an fp16 DVE add (2x mode) merges the halves.
        NSTAGE = 0
        with tc.tile_pool(name="uw", bufs=3) as uwp, \
             tc.tile_pool(name="psC", bufs=3, space="PSUM") as psC, \
             tc.tile_pool(name="u16p", bufs=1) as u16p, \
             tc.tile_pool(name="ob", bufs=2) as obp:
            def emit_c_mms(dt, stop):
                ut = uwp.tile([128, KT, 128], F8, name=f"ut{dt}", tag="ut")
                # u8 weights ride the sync queue: SP is idle in phase C while
                # ACT is still draining phase A epilogues
                nc.sync.dma_start(out=ut, in_=d_u8[:, :, :, :][dt])
                po = [psC.tile([128, 512], F32, name=f"po{dt}_{t}", tag=f"po{t}") for t in range(2)]
                for kk in range(KT // 2):
                    for t in range(2):
                        sl = slice(t * 512, (t + 1) * 512)
                        nc.tensor.matmul(
                            po[t], ut[:, 2 * kk:2 * kk + 2, :],
                            x8t[:, 2 * kk:2 * kk + 2, sl],
                            start=(kk == 0), stop=(stop and kk == KT // 2 - 1),
                            perf_mode=DR)
                return po

            def emit_c_epilogue(dt, po, nq=1, dve=False):
                # dve=True splits the scaled psum->fp16 copies across ACT
                # (t=0) and DVE (t=1) to shorten the kernel-exit drain
                o16 = obp.tile([128, TPC], F16, name=f"o16_{dt}", tag="o16")
                for t in range(2):
                    sl = slice(t * 512, (t + 1) * 512)
                    nc.tensor.matmul(po[t], vw[:, dt, :], zT16[:, sl],
                                     start=False, stop=True)
                    for q in range(nq):
                        qs = slice(t * 512 + q * (512 // nq),
                                   t * 512 + (q + 1) * (512 // nq))
                        qp = slice(q * (512 // nq), (q + 1) * (512 // nq))
                        if dve and t == 1:
                            nc.vector.tensor_scalar_mul(
                                o16[:, qs], po[t][:, qp], 1.0 / W8)
                        else:
                            nc.scalar.activation(
                                o16[:, qs], po[t][:, qp],
                                mybir.ActivationFunctionType.Copy,
                                scale=1.0 / W8)
                        nc.sync.dma_start(
                            out=d_out[dt * 128:(dt + 1) * 128, qs],
                            in_=o16[:, qs])

            pend = [emit_c_mms(dt, stop=False) for dt in range(3)]
            for tt in range(NTT):
                pz = psC.tile([L, 128], F32, name=f"pz{tt}", tag="pz", bufs=2)
                nc.tensor.transpose(pz, zt[:, tt, :], ident)
                nc.vector.tensor_copy(zT16[:, tt * 128:(tt + 1) * 128], pz)
            for dt in range(3):
                emit_c_epilogue(dt, pend[dt])
            for dt in range(3, NDT):
                po = emit_c_mms(dt, stop=False)
                emit_c_epilogue(dt, po, nq=2 if dt == NDT - 1 else 1,
                                dve=dt >= NDT - 2)

    nc.finalize()
    return nc


_NC_CACHE = {}


def _get_program():
    if "nc" not in _NC_CACHE:
        _NC_CACHE["nc"] = _build_program()
    return _NC_CACHE["nc"]


def make_in_maps(x, U_w, gate_w, ladder_w, V_w):
    """Host-side sharding + layout prep. Returns per-core input dicts."""
    x2 = np.ascontiguousarray(np.asarray(x, dtype=np.float32).reshape(TOKENS, D))

    def wtiles(w):
        # w: [out, in] fp32 -> tiles [dt][p][k][o] with
        # tile[dt, p, k, o] = w[dt*128+o, k*128+p]
        wT = w.T.astype(np.float32)                    # [d, o]
        a = wT.reshape(KT, 128, NDT, 128)              # [k, p, dt, o]
        return np.ascontiguousarray(a.transpose(2, 1, 0, 3))

    U_w = np.asarray(U_w, np.float32)
    gate_w = np.asarray(gate_w, np.float32)
    ladder_w = np.asarray(ladder_w, np.float32)
    V_w = np.asarray(V_w, np.float32)

    g_tiles = wtiles(gate_w)
    ghi_t = g_tiles.astype(np.float16)
    glo_t = (g_tiles - ghi_t.astype(np.float32)) * SC
    gl8_t = _to8(glo_t * W8)
    u8_t = _to8(wtiles(U_w) * W8)

    lwT = ladder_w.transpose(2, 0, 1).reshape(D, LK)   # [d, (l k)]
    lw_hi, lw_lo = _split16(lwT)
    # [p, k, lk] with element (p,k,lk) = lwT[k*128+p, lk]
    lwhi_t = np.ascontiguousarray(
        lw_hi.reshape(KT, 128, LK).transpose(1, 0, 2))
    lwp8_t = np.zeros((128, KT, 2, 32), dtype=mybir.dt.np(F8))
    lwp8_t[:, :, 0, :LK] = _to8(
        lw_hi.astype(np.float32).reshape(KT, 128, LK).transpose(1, 0, 2) * W8)
    lwp8_t[:, :, 1, :LK] = _to8(
        lw_lo.astype(np.float32).reshape(KT, 128, LK).transpose(1, 0, 2) * W8)

    vwT = np.ascontiguousarray(
        (V_w.T.reshape(L, NDT, 128) * W8).astype(np.float16))
    vsc_t = np.ascontiguousarray(
        V_w.reshape(NDT, 128, L).transpose(1, 0, 2)).astype(np.float32)

    in_maps = []
    for c in range(NCORES):
        shard = x2[c * TPC:(c + 1) * TPC]              # [TPC, D]
        xT = np.ascontiguousarray(shard.T)             # [D, TPC]
        x_hi, x_lo = _split16(xT)
        in_maps.append({
            "xhi": np.ascontiguousarray(x_hi.reshape(KT, 128, TPC)),
            "xlo": np.ascontiguousarray(x_lo.reshape(KT, 128, TPC)),
            "ghi": ghi_t, "gl8": gl8_t, "u8": u8_t,
            "lwhi": lwhi_t, "lwp8": lwp8_t, "vwT": vwT, "vsc": vsc_t,
        })
    return in_maps


def assemble_output(results):
    parts = [results[c]["outT"].astype(np.float32).T
             for c in range(NCORES)]                         # [TPC, D] each
    out = np.concatenate(parts, axis=0)                      # [TOKENS, D]
    return np.ascontiguousarray(out.reshape(4, 2048, D).astype(np.float32))


def kernel(x, U_w, gate_w, ladder_w, V_w):
    nc = _get_program()
    in_maps = make_in_maps(x, U_w, gate_w, ladder_w, V_w)
    res = run_bass_kernel_spmd(nc, in_maps, list(range(NCORES)))
    return assemble_output(res.results)


if __name__ == "__main__":
    rng = np.random.default_rng(0)
    x = rng.normal(0, 1, (4, 2048, D)).astype(np.float32)
    s = 1.0 / np.sqrt(D)
    U_w = rng.uniform(-s, s, (D, D)).astype(np.float32)
    gate_w = rng.uniform(-s, s, (D, D)).astype(np.float32)
    ladder_w = rng.uniform(-s, s, (L, DEPTH + 1, D)).astype(np.float32)
    V_w = rng.uniform(-1 / np.sqrt(L), 1 / np.sqrt(L), (D, L)).astype(np.float32)
    out = kernel(x=x, U_w=U_w, gate_w=gate_w, ladder_w=ladder_w, V_w=V_w)
    print("out", out.shape, out.dtype, np.abs(out).max())


# revision 63
# speedup vs baseline: 1.0106x; 1.0050x over previous
"""Trainium2 Bass kernel for nn_Cffn_68478958568093 (dense_mlp).

out = x @ U_w.T + V(z),  z = a0 + continued_fraction(a[..,1:]),
a = (sigmoid(x @ gate_w.T) * x) @ ladder_w.T

Distribution: data-parallel over the 8192 tokens across 8 NeuronCores
(1024 tokens/core), weights replicated.  All on-chip compute is done in
feature-major (transposed) layout; the host transposes per-core shards in
and the final output back.

Precision budget (tolerance is rel 2e-2 of absmax 37.9 => abs ~0.76; the
continued fraction amplifies errors in `a` by ~1e4, and flips sign when
a-noise exceeds the min unguarded |1+f| ~ 4e-4 -- emulated margins in
emul2.py):
 - gate GEMM: fp16 hi*hi pass + fp8 DoubleRow chain for the w_lo * x
   cross term (weights prescaled x32).  Dropping only the w_hi*x_lo
   term; emulated max out err 0.232 (rel 6.1e-3, 3.3x margin).
 - ladder GEMM (K=2048 -> 18): fp16 hi*hi pass + one fp8 DoubleRow pass
   computing both cross terms (lw_hi*g_lo + lw_lo*g_hi), operands
   prescaled x32 to stay out of e4m3 subnormals => e_ladder ~ 4e-6.
 - U GEMM: tolerance-loose, fp8e4 DoubleRow (0.5 cyc/row, 256-row
   contraction per instr), weights prescaled x32; psum is divided by 32
   in the ACT-engine epilogue.
 - V (K=3): fp16 matmul appended to each U psum accumulation chain
   (V.T prescaled x32 to match), so the epilogue is a single ACT copy
   with scale 1/32 and there is no z broadcast or DVE FMA chain.

 - output written fp16 (adds <=2.4e-4 relative; halves the exit DMA).

PE work: gate fp16 109us + gate cross DR 27us, ladder 10us, U 27us,
V 7us => 182us busy; CoreSim ~199us total (lead-in, phase-boundary
pool barriers, exit drain).  Measured on HW: max abs err 0.232090
(rel 6.13e-3 vs the 2e-2 gate), CoreSim cost model 197720 ns vs the
478825 ns baseline (2.42x).
"""

import numpy as np
from contextlib import ExitStack

import concourse.bass as bass
import concourse.bacc as bacc
import concourse.mybir as mybir
import concourse.tile as tile
from concourse.bass_utils import run_bass_kernel_spmd
from concourse.masks import make_identity

NCORES = 8
D = 2048
TOKENS = 4 * 2048
TPC = TOKENS // NCORES      # tokens per core = 1024
KT = D // 128               # 16 contraction chunks
NDT = D // 128              # 16 output-row tiles
NTT = TPC // 128            # 8 token tiles of 128
L = 3
DEPTH = 5
LK = L * (DEPTH + 1)        # 18
EPS = 0.01
SC = 2048.0                 # 2^11 hi/lo split scale
W8 = 32.0                   # fp8 weight prescale (avoids e4m3 subnormals)
F16 = mybir.dt.float16
F32 = mybir.dt.float32
F8 = mybir.dt.float8e4
AOP = mybir.AluOpType
DR = mybir.MatmulPerfMode.DoubleRow


def _split16(a):
    """fp32 array -> (hi fp16, lo' fp16) with lo' = (a - hi) * 2048."""
    hi = a.astype(np.float16)
    lo = ((a - hi.astype(np.float32)) * SC).astype(np.float16)
    return hi, lo


def _to8(a):
    return np.clip(np.asarray(a, np.float32), -240.0, 240.0).astype(
        mybir.dt.np(F8))


def _build_program():
    nc = bacc.Bacc()

    def dp(name, shape, dt, out=False):
        return nc.declare_dram_parameter(name, list(shape), dt, isOutput=out)

    d_xhi = dp("xhi", [KT, 128, TPC], F16)
    d_xlo = dp("xlo", [KT, 128, TPC], F16)
    d_ghi = dp("ghi", [NDT, 128, KT, 128], F16)   # [dt][p][k][o]
    d_gl8 = dp("gl8", [NDT, 128, KT, 128], F8)    # gate_lo16 * 32, same tiling
    d_u8 = dp("u8", [NDT, 128, KT, 128], F8)      # U_w * 32, same tiling
    d_lwhi = dp("lwhi", [128, KT, LK], F16)
    # (lw_hi*32, lw_lo*32) pairs; last dim padded 18->32 so the DoubleRow
    # LDWEIGHTS subtile step is a multiple of 16 (ISA requirement)
    d_lwp8 = dp("lwp8", [128, KT, 2, 32], F8)
    d_vw = dp("vwT", [L, NDT, 128], F16)          # V_w.T * 32
    d_vsc = dp("vsc", [128, NDT, L], F32)         # V_w rows by partition
    d_out = dp("outT", [D, TPC], F16, out=True)

    with tile.TileContext(nc) as tc, ExitStack() as ctx:
        persist = ctx.enter_context(tc.tile_pool(name="persist", bufs=1))
        gwp = ctx.enter_context(tc.tile_pool(name="gw", bufs=2))
        drp = ctx.enter_context(tc.tile_pool(name="drs", bufs=1, space="DRAM"))

        # dt0's gate weights load at the head of both DMA queues so the PE
        # can start within ~1us; the first xhi tile is split for the same
        # reason.  x8 (fp8 x for the DoubleRow chains) is cast on-chip by
        # the DVE as each xhi tile lands -- no DMA on the critical lead-in.
        gh0 = gwp.tile([128, KT, 128], F16, tag="gh")
        nc.sync.dma_start(out=gh0[:, 0:2, :], in_=d_ghi[0][:, 0:2, :])
        # the bulk of dt0's weights rides the scalar queue so it does not
        # stall the xhi stream on sync (k=2 is needed ~3.4us in)
        nc.scalar.dma_start(out=gh0[:, 2:KT, :], in_=d_ghi[0][:, 2:KT, :])
        gl80 = gwp.tile([128, KT, 128], F8, tag="gl8")
        nc.scalar.dma_start(out=gl80, in_=d_gl8[:, :, :, :][0])
        x8t = persist.tile([128, KT, TPC], F8, tag="x8t")
        xhi, xlo = [], []
        for k in range(KT):
            t = persist.tile([128, TPC], F16, tag=f"xhi{k}")
            if k == 0:
                nc.sync.dma_start(out=t[:, 0:512], in_=d_xhi[0][:, 0:512])
                nc.sync.dma_start(out=t[:, 512:TPC], in_=d_xhi[0][:, 512:TPC])
            else:
                nc.sync.dma_start(out=t, in_=d_xhi[:, :, :][k])
            nc.vector.tensor_copy(x8t[:, k, :], t)
            xhi.append(t)
        for k in range(KT):
            t = persist.tile([128, TPC], F16, tag=f"xlo{k}")
            nc.sync.dma_start(out=t, in_=d_xlo[:, :, :][k])
            xlo.append(t)

        lwhi = persist.tile([128, KT, LK], F16, tag="lwhi")
        lwp8 = persist.tile([128, KT, 2, 32], F8, tag="lwp8")
        vw = persist.tile([L, NDT, 128], F16, tag="vw")
        vsc = persist.tile([128, NDT, L], F32, tag="vsc")

        ident = persist.tile([128, 128], F32, tag="ident")
        make_identity(nc, ident)

        ghi_g = []                  # gated_x hi fp16, by row tile
        for k in range(KT):
            ghi_g.append(persist.tile([128, TPC], F16, name=f"gghi{k}", tag=f"gghi{k}"))
        # fp8 (lo, hi) pairs of gated_x for the ladder cross-term DoubleRow
        gq8 = persist.tile([128, KT, 2, TPC], F8, tag="gq8")
        zt = persist.tile([128, NTT, L], F32, tag="zt")
        zT16 = persist.tile([L, TPC], F16, tag="zT16")

        # ---------------- Phase A: gated_x = sigmoid(x @ gate_w.T) * x -----
        # fp16 hi*hi pass + fp8 DoubleRow chain for the w_lo*x cross term;
        # epilogue reconstructs x in fp32, applies the sigmoid gate, and
        # stores gated_x both as fp16 hi and as fp8 (lo, hi) pairs.
        with tc.tile_pool(name="psA", bufs=2, space="PSUM") as psA, \
             tc.tile_pool(name="epi", bufs=1) as epi:
            for dt in range(NDT):
                if dt == 0:
                    gh, gl8 = gh0, gl80
                else:
                    gh = gwp.tile([128, KT, 128], F16, tag="gh")
                    nc.scalar.dma_start(out=gh, in_=d_ghi[:, :, :, :][dt])
                    gl8 = gwp.tile([128, KT, 128], F8, tag="gl8")
                    nc.scalar.dma_start(out=gl8, in_=d_gl8[:, :, :, :][dt])
                if dt == 1:
                    # small phase-B/C constants ride behind dt1's weights
                    nc.scalar.dma_start(out=lwhi, in_=d_lwhi[:, :, :])
                    nc.scalar.dma_start(out=lwp8, in_=d_lwp8[:, :, :, :])
                    nc.scalar.dma_start(out=vw, in_=d_vw[:, :, :])
                    nc.scalar.dma_start(out=vsc, in_=d_vsc[:, :, :])

                pm = [psA.tile([128, 512], F32, name=f"pm{dt}_{t}", tag=f"pm{t}") for t in range(2)]
                pc = [psA.tile([128, 512], F32, name=f"pc{dt}_{t}", tag=f"pc{t}") for t in range(2)]
                for k in range(KT):
                    first, last = k == 0, k == KT - 1
                    for t in range(2):
                        sl = slice(t * 512, (t + 1) * 512)
                        nc.tensor.matmul(pm[t], gh[:, k, :], xhi[k][:, sl],
                                         start=first, stop=last)
                for kk in range(KT // 2):
                    first, last = kk == 0, kk == KT // 2 - 1
                    for t in range(2):
                        sl = slice(t * 512, (t + 1) * 512)
                        nc.tensor.matmul(
                            pc[t], gl8[:, 2 * kk:2 * kk + 2, :],
                            x8t[:, 2 * kk:2 * kk + 2, sl],
                            start=first, stop=last, perf_mode=DR)

                # DVE may read only one PSUM operand per op: ACT stages the
                # scaled cross psum to SBUF, DVE adds the hi*hi psum.
                c32 = epi.tile([128, TPC], F32, tag="c32")
                y32 = epi.tile([128, TPC], F32, tag="y32")
                sig = epi.tile([128, TPC], F32, tag="sig")
                for t in range(2):
                    sl = slice(t * 512, (t + 1) * 512)
                    nc.scalar.activation(c32[:, sl], pc[t],
                                         mybir.ActivationFunctionType.Copy,
                                         scale=1.0 / (SC * W8))
                    nc.vector.tensor_add(y32[:, sl], c32[:, sl], pm[t])
                    nc.scalar.activation(sig[:, sl], y32[:, sl],
                                         mybir.ActivationFunctionType.Sigmoid)
                x32 = epi.tile([128, TPC], F32, tag="x32")
                nc.vector.scalar_tensor_tensor(
                    out=x32, in0=xlo[dt], scalar=1.0 / SC, in1=xhi[dt],
                    op0=AOP.mult, op1=AOP.add)
                g32 = epi.tile([128, TPC], F32, tag="g32")
                nc.vector.tensor_mul(g32, sig, x32)
                nc.vector.tensor_copy(ghi_g[dt], g32)
                d32 = epi.tile([128, TPC], F32, tag="d32")
                nc.vector.scalar_tensor_tensor(
                    out=d32, in0=ghi_g[dt], scalar=-1.0, in1=g32,
                    op0=AOP.mult, op1=AOP.add)
                # fp8 lo (scaled by SC) and hi copies for the ladder cross
                nc.vector.tensor_scalar_mul(gq8[:, dt, 0, :], d32, SC)
                nc.vector.tensor_copy(gq8[:, dt, 1, :], ghi_g[dt])

        # ---------------- Phase B: a = gated @ lw.T ; CF ; z ---------------
        # hi*hi in fp16 + both cross terms in one fp8 DoubleRow chain.
        with tc.tile_pool(name="cfb", bufs=1) as cfb, \
             tc.tile_pool(name="psB", bufs=2, space="PSUM") as psB:
            a32 = cfb.tile([LK, TPC], F32, tag="a32")
            for t in range(2):
                sl = slice(t * 512, (t + 1) * 512)
                pam = psB.tile([LK, 512], F32, tag="pam")
                pac = psB.tile([LK, 512], F32, tag="pac")
                for k in range(KT):
                    first, last = k == 0, k == KT - 1
                    nc.tensor.matmul(pam, lwhi[:, k, :], ghi_g[k][:, sl],
                                     start=first, stop=last)
                for k in range(KT):
                    first, last = k == 0, k == KT - 1
                    nc.tensor.matmul(pac, lwp8[:, k, :, 0:LK], gq8[:, k, :, sl],
                                     start=first, stop=last, perf_mode=DR)
                nc.vector.tensor_copy(a32[:, sl], pam)
                nc.vector.scalar_tensor_tensor(
                    out=a32[:, sl], in0=pac, scalar=1.0 / (SC * W8),
                    in1=a32[:, sl], op0=AOP.mult, op1=AOP.add)

            # transpose a to token-major [128, tt, l, k]
            at = cfb.tile([128, NTT, L, DEPTH + 1], F32, tag="at")
            for tt in range(NTT):
                pt = psB.tile([128, LK], F32, tag="pt")
                nc.tensor.transpose(
                    pt, a32[:, tt * 128:(tt + 1) * 128], ident[:LK, :LK])
                nc.vector.tensor_copy(
                    at[:, tt, :, :].rearrange("p l k -> p (l k)"), pt)

            # continued fraction with eps-guarded denominators
            f = cfb.tile([128, NTT, L], F32, tag="f")
            t1 = cfb.tile([128, NTT, L], F32, tag="t1")
            dc = cfb.tile([128, NTT, L], F32, tag="dc")
            msk = cfb.tile([128, NTT, L], mybir.dt.uint8, tag="msk")
            rc = cfb.tile([128, NTT, L], F32, tag="rc")
            nc.vector.tensor_copy(f, at[:, :, :, DEPTH])
            for kk in range(DEPTH - 1, 0, -1):
                nc.vector.tensor_scalar(out=t1, in0=f, scalar1=1.0,
                                        scalar2=EPS, op0=AOP.add, op1=AOP.max)
                nc.vector.tensor_scalar(out=dc, in0=f, scalar1=1.0,
                                        scalar2=-EPS, op0=AOP.add, op1=AOP.min)
                nc.vector.tensor_scalar(out=msk, in0=f, scalar1=1.0,
                                        scalar2=0.0, op0=AOP.add, op1=AOP.is_ge)
                nc.vector.copy_predicated(dc, msk, t1)
                nc.vector.reciprocal(rc, dc)
                nc.vector.tensor_mul(f, at[:, :, :, kk], rc)
            nc.vector.tensor_add(zt, at[:, :, :, 0], f)

        # ---------------- Phase C: out = (32*(x@U.T) + 32*(z@V.T)) / 32 ----
        # U in fp8 DoubleRow; V appended as a K=3 fp16 matmul on the same
        # psum chain once z is ready; epilogue is one ACT copy w/ scale.
        # The first NSTAGE chains stop without V and stage scaled fp16
        # U-results to SBUF, freeing psum so the PE streams through the
        # continued-fraction latency window; their V matmuls land in fresh
        # psum later and an fp16 DVE add (2x mode) merges the halves.
        NSTAGE = 0
        with tc.tile_pool(name="uw", bufs=3) as uwp, \
             tc.tile_pool(name="psC", bufs=3, space="PSUM") as psC, \
             tc.tile_pool(name="u16p", bufs=1) as u16p, \
             tc.tile_pool(name="ob", bufs=2) as obp:
            def emit_c_mms(dt, stop):
                ut = uwp.tile([128, KT, 128], F8, name=f"ut{dt}", tag="ut")
                # u8 weights ride the sync queue: SP is idle in phase C while
                # ACT is still draining phase A epilogues
                nc.sync.dma_start(out=ut, in_=d_u8[:, :, :, :][dt])
                po = [psC.tile([128, 512], F32, name=f"po{dt}_{t}", tag=f"po{t}") for t in range(2)]
                for kk in range(KT // 2):
                    for t in range(2):
                        sl = slice(t * 512, (t + 1) * 512)
                        nc.tensor.matmul(
                            po[t], ut[:, 2 * kk:2 * kk + 2, :],
                            x8t[:, 2 * kk:2 * kk + 2, sl],
                            start=(kk == 0), stop=(stop and kk == KT // 2 - 1),
                            perf_mode=DR)
                return po

            def emit_c_epilogue(dt, po, nq=1, dve=False):
                # dve=True splits the scaled psum->fp16 copies across ACT
                # (t=0) and DVE (t=1) to shorten the kernel-exit drain
                o16 = obp.tile([128, TPC], F16, name=f"o16_{dt}", tag="o16")
                for t in range(2):
                    sl = slice(t * 512, (t + 1) * 512)
                    nc.tensor.matmul(po[t], vw[:, dt, :], zT16[:, sl],
                                     start=False, stop=True)
                    for q in range(nq):
                        qs = slice(t * 512 + q * (512 // nq),
                                   t * 512 + (q + 1) * (512 // nq))
                        qp = slice(q * (512 // nq), (q + 1) * (512 // nq))
                        if dve and t == 1:
                            nc.vector.tensor_scalar_mul(
                                o16[:, qs], po[t][:, qp], 1.0 / W8)
                        else:
                            nc.scalar.activation(
                                o16[:, qs], po[t][:, qp],
                                mybir.ActivationFunctionType.Copy,
                                scale=1.0 / W8)
                        nc.sync.dma_start(
                            out=d_out[dt * 128:(dt + 1) * 128, qs],
                            in_=o16[:, qs])

            pend = [emit_c_mms(dt, stop=False) for dt in range(3)]
            for tt in range(NTT):
                pz = psC.tile([L, 128], F32, name=f"pz{tt}", tag="pz", bufs=2)
                nc.tensor.transpose(pz, zt[:, tt, :], ident)
                nc.vector.tensor_copy(zT16[:, tt * 128:(tt + 1) * 128], pz)
            for dt in range(3):
                emit_c_epilogue(dt, pend[dt])
            for dt in range(3, NDT):
                po = emit_c_mms(dt, stop=False)
                # half-grain (not quarter) epilogue on the last tile: each
                # dma_start costs ~0.5us serially on the queue at the exit
                emit_c_epilogue(dt, po, nq=1, dve=dt >= NDT - 2)

    nc.finalize()
    return nc


_NC_CACHE = {}


def _get_program():
    if "nc" not in _NC_CACHE:
        _NC_CACHE["nc"] = _build_program()
    return _NC_CACHE["nc"]


def make_in_maps(x, U_w, gate_w, ladder_w, V_w):
    """Host-side sharding + layout prep. Returns per-core input dicts."""
    x2 = np.ascontiguousarray(np.asarray(x, dtype=np.float32).reshape(TOKENS, D))

    def wtiles(w):
        # w: [out, in] fp32 -> tiles [dt][p][k][o] with
        # tile[dt, p, k, o] = w[dt*128+o, k*128+p]
        wT = w.T.astype(np.float32)                    # [d, o]
        a = wT.reshape(KT, 128, NDT, 128)              # [k, p, dt, o]
        return np.ascontiguousarray(a.transpose(2, 1, 0, 3))

    U_w = np.asarray(U_w, np.float32)
    gate_w = np.asarray(gate_w, np.float32)
    ladder_w = np.asarray(ladder_w, np.float32)
    V_w = np.asarray(V_w, np.float32)

    g_tiles = wtiles(gate_w)
    ghi_t = g_tiles.astype(np.float16)
    glo_t = (g_tiles - ghi_t.astype(np.float32)) * SC
    gl8_t = _to8(glo_t * W8)
    u8_t = _to8(wtiles(U_w) * W8)

    lwT = ladder_w.transpose(2, 0, 1).reshape(D, LK)   # [d, (l k)]
    lw_hi, lw_lo = _split16(lwT)
    # [p, k, lk] with element (p,k,lk) = lwT[k*128+p, lk]
    lwhi_t = np.ascontiguousarray(
        lw_hi.reshape(KT, 128, LK).transpose(1, 0, 2))
    lwp8_t = np.zeros((128, KT, 2, 32), dtype=mybir.dt.np(F8))
    lwp8_t[:, :, 0, :LK] = _to8(
        lw_hi.astype(np.float32).reshape(KT, 128, LK).transpose(1, 0, 2) * W8)
    lwp8_t[:, :, 1, :LK] = _to8(
        lw_lo.astype(np.float32).reshape(KT, 128, LK).transpose(1, 0, 2) * W8)

    vwT = np.ascontiguousarray(
        (V_w.T.reshape(L, NDT, 128) * W8).astype(np.float16))
    vsc_t = np.ascontiguousarray(
        V_w.reshape(NDT, 128, L).transpose(1, 0, 2)).astype(np.float32)

    in_maps = []
    for c in range(NCORES):
        shard = x2[c * TPC:(c + 1) * TPC]              # [TPC, D]
        xT = np.ascontiguousarray(shard.T)             # [D, TPC]
        x_hi, x_lo = _split16(xT)
        in_maps.append({
            "xhi": np.ascontiguousarray(x_hi.reshape(KT, 128, TPC)),
            "xlo": np.ascontiguousarray(x_lo.reshape(KT, 128, TPC)),
            "ghi": ghi_t, "gl8": gl8_t, "u8": u8_t,
            "lwhi": lwhi_t, "lwp8": lwp8_t, "vwT": vwT, "vsc": vsc_t,
        })
    return in_maps


def assemble_output(results):
    parts = [results[c]["outT"].astype(np.float32).T
             for c in range(NCORES)]                         # [TPC, D] each
    out = np.concatenate(parts, axis=0)                      # [TOKENS, D]
    return np.ascontiguousarray(out.reshape(4, 2048, D).astype(np.float32))


def kernel(x, U_w, gate_w, ladder_w, V_w):
    nc = _get_program()
    in_maps = make_in_maps(x, U_w, gate_w, ladder_w, V_w)
    res = run_bass_kernel_spmd(nc, in_maps, list(range(NCORES)))
    return assemble_output(res.results)


if __name__ == "__main__":
    rng = np.random.default_rng(0)
    x = rng.normal(0, 1, (4, 2048, D)).astype(np.float32)
    s = 1.0 / np.sqrt(D)
    U_w = rng.uniform(-s, s, (D, D)).astype(np.float32)
    gate_w = rng.uniform(-s, s, (D, D)).astype(np.float32)
    ladder_w = rng.uniform(-s, s, (L, DEPTH + 1, D)).astype(np.float32)
    V_w = rng.uniform(-1 / np.sqrt(L), 1 / np.sqrt(L), (D, L)).astype(np.float32)
    out = kernel(x=x, U_w=U_w, gate_w=gate_w, ladder_w=ladder_w, V_w=V_w)
    print("out", out.shape, out.dtype, np.abs(out).max())
